# revision 5
# baseline (speedup 1.0000x reference)
"""Trainium2 Bass kernel for the LSTM seq2seq autoencoder (layout B, v2).

Strategy:
  - Data-parallel over batch: B=512 -> 64 rows per core on 8 cores.
  - Gates-on-partitions layout: gate preactivations live in PSUM banks
    [128, 512] = 8 chunks x 64 batch cols, chunk order [i0 i1 f0 f1 o0 o1 g0 g1].
  - fp8e4 weights scaled x16 (normal range); h carried at 1/16 scale via a
    fused scalar_tensor_tensor h-mul. FWL at 4 elem/cycle halves the
    LDWEIGHTS pacing of the per-step matmul burst (53ns -> ~30ns/pair).
  - Encoder bias folded into x-weight row 64 (xp row 64 is constant): no
    bias matmuls in the encoder; x matmuls open the PSUM banks.
  - Encoder length masking: c frozen by forcing i -> -BIG, f -> +BIG via the
    mbar row of xp; o captured at the freeze step by a single in-place DVE
    copy_predicated(o_acc, e_mask, o_t) with host-precomputed e-mask tiles
    (no PE transpose, no ACT copy).
  - Decoder feedback folded: W_comb = Whh + Wih_dec @ out_W.
  - y = out_W @ h computed in-loop: 2 small matmuls per step accumulate into
    a persistent PSUM bank (8 steps per group, opened by the k0 matmul);
    evacuated by a DVE tensor_scalar (+out_b) in the PE window + one DMA,
    keeping the ACT engine free for the recurrence chain.
"""

import numpy as np
import ml_dtypes
from contextlib import ExitStack

import concourse.bass as bass
import concourse.bacc as bacc
import concourse.mybir as mybir
import concourse.tile as tile
from concourse.tile import add_dep_helper
from concourse.bass_utils import run_bass_kernel_spmd

B, T, D, H = 512, 512, 64, 256
G4 = 4 * H  # 1024
NCORES = 8
BL = B // NCORES  # 64
TDEC = T - 1      # 511 decoder steps
BIG = 192.0       # freeze logit magnitude (exactly representable in fp8e4m3)
WS = 16.0         # fp8 weight scale; h carried at 1/WS
F32 = mybir.dt.float32
BF16 = mybir.dt.bfloat16
F8 = mybir.dt.float8e4
BF = ml_dtypes.bfloat16
F8NP = mybir.dt.np(F8)

_PROGRAM = None
LAST_RESULTS = None

# chunk order on the 512 free cols: [i0 i1 f0 f1 o0 o1 g0 g1]
# torch gate rows: i=[0,256) f=[256,512) g=[512,768) o=[768,1024)
CHUNK_ROWS = [(0, 128), (128, 256), (256, 384), (384, 512),
              (768, 896), (896, 1024), (512, 640), (640, 768)]

Sig = mybir.ActivationFunctionType.Sigmoid
Tanh = mybir.ActivationFunctionType.Tanh
MUL = mybir.AluOpType.mult
ADD = mybir.AluOpType.add


def build_program(t_enc=T, t_dec=TDEC):
    nc = bacc.Bacc(None, target_bir_lowering=False)
    f = F32
    xp_d = nc.dram_tensor("xp", [t_enc, 66, BL], BF16, kind="ExternalInput")
    x0p_d = nc.dram_tensor("x0p", [66, BL], BF16, kind="ExternalInput")
    wxenc_d = nc.dram_tensor("wxenc", [66, 8, 128], BF16, kind="ExternalInput")
    wxdec_d = nc.dram_tensor("wxdec", [66, 8, 128], BF16, kind="ExternalInput")
    whhenc_d = nc.dram_tensor("whhenc", [128, 2, 8, 128], BF16, kind="ExternalInput")
    wcomb_d = nc.dram_tensor("wcomb", [128, 2, 8, 128], BF16, kind="ExternalInput")
    whhdec_d = nc.dram_tensor("whhdec", [128, 2, 8, 128], BF16, kind="ExternalInput")
    biasA_d = nc.dram_tensor("biasA", [4, 128], BF16, kind="ExternalInput")
    biasB_d = nc.dram_tensor("biasB", [2, 128], BF16, kind="ExternalInput")
    biasC_d = nc.dram_tensor("biasC", [2, 128], BF16, kind="ExternalInput")
    blockones_d = nc.dram_tensor("blockones", [8, 512], BF16, kind="ExternalInput")
    emask_d = nc.dram_tensor("emask", [t_enc, 128, 128], BF16, kind="ExternalInput")
    outwT_d = nc.dram_tensor("outwT", [128, 2, 128], BF16, kind="ExternalInput")
    outb_d = nc.dram_tensor("outb", [D, 1], F32, kind="ExternalInput")
    yt_d = nc.dram_tensor("yt", [D, t_dec + 1, BL], F32, kind="ExternalOutput")

    with ExitStack() as ctx:
        tc = ctx.enter_context(tile.TileContext(nc))
        singles = ctx.enter_context(tc.tile_pool(name="singles", bufs=1))
        xpool = ctx.enter_context(tc.tile_pool(name="xpool", bufs=6))
        epool = ctx.enter_context(tc.tile_pool(name="epool", bufs=4))
        oap = ctx.enter_context(tc.tile_pool(name="oap", bufs=2))
        work = ctx.enter_context(tc.tile_pool(name="work", bufs=3))
        hpool = ctx.enter_context(tc.tile_pool(name="hpool", bufs=2))
        cpool = ctx.enter_context(tc.tile_pool(name="cpool", bufs=2))
        gpool = ctx.enter_context(
            tc.tile_pool(name="gpool", bufs=3, space=bass.MemorySpace.PSUM))
        gbc = ctx.enter_context(
            tc.tile_pool(name="gbc", bufs=2, space=bass.MemorySpace.PSUM))
        tpp = ctx.enter_context(
            tc.tile_pool(name="tpp", bufs=1, space=bass.MemorySpace.PSUM))

        # ---- persistent constants ----
        s_wxenc = singles.tile([66, 8, 128], BF16)
        nc.sync.dma_start(s_wxenc, wxenc_d[:, :, :])
        s_wxdec = singles.tile([66, 8, 128], BF16)
        nc.sync.dma_start(s_wxdec, wxdec_d[:, :, :])
        s_whhenc = singles.tile([128, 2, 8, 128], BF16)
        nc.sync.dma_start(s_whhenc, whhenc_d[:, :, :, :])
        s_whhdec = singles.tile([128, 2, 8, 128], BF16)
        nc.sync.dma_start(s_whhdec, whhdec_d[:, :, :, :])
        s_wcomb = singles.tile([128, 2, 8, 128], BF16)
        nc.sync.dma_start(s_wcomb, wcomb_d[:, :, :, :])
        s_biasA = singles.tile([4, 128], BF16)
        nc.sync.dma_start(s_biasA, biasA_d[:, :])
        s_biasB = singles.tile([2, 128], BF16)
        nc.sync.dma_start(s_biasB, biasB_d[:, :])
        s_biasC = singles.tile([2, 128], BF16)
        nc.sync.dma_start(s_biasC, biasC_d[:, :])
        s_bonesA = singles.tile([4, 256], BF16)
        nc.sync.dma_start(s_bonesA, blockones_d[0:4, 0:256])
        s_bonesBC = singles.tile([2, 128], BF16)
        nc.sync.dma_start(s_bonesBC, blockones_d[4:6, 256:384])
        s_outwT = singles.tile([128, 2, 128], BF16)
        nc.sync.dma_start(s_outwT, outwT_d[:, :, :])
        s_outb = singles.tile([D, 1], f)
        nc.sync.dma_start(s_outb, outb_d[:, :])
        s_x0p = singles.tile([66, BL], BF16)
        nc.sync.dma_start(s_x0p, x0p_d[:, :])

        # ---- initial state ----
        c_prev = singles.tile([128, 2, BL], f, tag="c0")
        nc.vector.memset(c_prev, 0.0)
        hT_i0 = singles.tile([128, BL], BF16, tag="hi0")
        nc.vector.memset(hT_i0, 0.0)
        hT_i1 = singles.tile([128, BL], BF16, tag="hi1")
        nc.vector.memset(hT_i1, 0.0)
        hT_prev = (hT_i0, hT_i1)
        o_acc = singles.tile([128, 128], BF16, tag="oacc0")
        nc.vector.memset(o_acc, 0.0)

        def chain(insts):
            for a, b in zip(insts[1:], insts[:-1]):
                add_dep_helper(a.ins, b.ins, sync=False, reason="pe-order")

        # gate chunk m -> (bank, col offset): A=i,f (m0-3), B=g (m6,7), C=o (m4,5)
        def bank_slice(psA, psB, psC, m):
            if m < 4:
                return psA[:, 64 * m:64 * m + 64]
            if m >= 6:
                return psB[:, 64 * (m - 6):64 * (m - 6) + 64]
            return psC[:, 64 * (m - 4):64 * (m - 4) + 64]

        def gate_mms(psA, psB, psC, whh, xlhs=None, xrhs=None, with_bias=False):
            """All matmuls of one step. Gates split across three PSUM banks
            so each ACT read waits only on its own bank's writers. The banks
            are opened either by the bias matmuls (decoder steps >= 1) or by
            the x matmuls (bias folded into x-weight row 64). h MMs go
            bank-A-first, k-interleaved per bank group; the explicit chain
            pins the scheduler to this PE order."""
            mms = []
            if with_bias:
                mms += [
                    nc.tensor.matmul(psA, s_biasA, s_bonesA,
                                     start=True, stop=False, skip_group_check=True),
                    nc.tensor.matmul(psB, s_biasB, s_bonesBC,
                                     start=True, stop=False, skip_group_check=True),
                    nc.tensor.matmul(psC, s_biasC, s_bonesBC,
                                     start=True, stop=False, skip_group_check=True),
                ]
            if xlhs is not None:
                for m in (0, 1, 2, 3, 6, 7, 4, 5):
                    # start=True clears the WHOLE bank: only the first
                    # writer of each bank may set it
                    st = (not with_bias) and m in (0, 6, 4)
                    mms.append(nc.tensor.matmul(bank_slice(psA, psB, psC, m),
                                                xlhs[:, m, :], xrhs,
                                                start=st, stop=False,
                                                skip_group_check=True))
            for ms in ((0, 1, 2, 3), (6, 7), (4, 5)):
                for k in (0, 1):
                    for m in ms:
                        mms.append(nc.tensor.matmul(
                            bank_slice(psA, psB, psC, m),
                            whh[:, k, m, :], hT_prev[k],
                            start=False, stop=(k == 1),
                            skip_group_check=True))
            chain(mms)
            return mms[-1]

        def cell(psA, psB, psC):
            """LSTM cell elementwise phase. Updates c_prev/hT_prev.
            Returns (o_t tile, last h-mul instruction)."""
            nonlocal c_prev, hT_prev
            if_t = work.tile([128, 256], BF16, tag="ift")
            nc.scalar.activation(if_t, psA, Sig)
            g_t = work.tile([128, 128], BF16, tag="gt")
            nc.scalar.activation(g_t, psB, Tanh)
            o_t = work.tile([128, 128], BF16, tag="ot")
            nc.scalar.activation(o_t, psC, Sig)
            c_new = cpool.tile([128, 2, BL], f, tag="c")
            tct = work.tile([128, 2, BL], BF16, tag="tct")
            hT_new = (hpool.tile([128, BL], BF16, tag="hT0", name="hT0"),
                      hpool.tile([128, BL], BF16, tag="hT1", name="hT1"))
            last_mul = None
            for k in (0, 1):
                sl = slice(64 * k, 64 * k + 64)
                fc = work.tile([128, BL], f, tag=f"fc{k}")
                nc.vector.tensor_mul(fc, if_t[:, 128 + 64 * k:192 + 64 * k],
                                     c_prev[:, k, :])
                ig = work.tile([128, BL], f, tag=f"ig{k}")
                nc.vector.tensor_mul(ig, if_t[:, sl], g_t[:, sl])
                nc.vector.tensor_add(c_new[:, k, :], fc, ig)
                nc.scalar.activation(tct[:, k, :], c_new[:, k, :], Tanh)
                # h stored at 1/WS to cancel the x16 fp8 weight scale
                last_mul = nc.vector.scalar_tensor_tensor(
                    hT_new[k], o_t[:, sl], 1.0 / WS, tct[:, k, :], MUL, MUL)
            c_prev = c_new
            hT_prev = hT_new
            return o_t, last_mul

        # ================= ENCODER =================
        for t in range(t_enc):
            xp_t = xpool.tile([66, BL], BF16, tag="xp")
            nc.sync.dma_start(xp_t, xp_d[t, :, :])
            e_t = epool.tile([128, 128], BF16, tag="et")
            nc.sync.dma_start(e_t, emask_d[t, :, :])
            psA = gpool.tile([128, 256], f, tag="gA")
            psB = gbc.tile([128, 128], f, tag="gB")
            psC = gbc.tile([128, 128], f, tag="gC")
            gate_mms(psA, psB, psC, s_whhenc, xlhs=s_wxenc, xrhs=xp_t)
            o_t, last_mul = cell(psA, psB, psC)
            # capture o at the freeze step on the otherwise-idle GPSIMD
            om = work.tile([128, 128], BF16, tag="om")
            nc.gpsimd.tensor_mul(om, o_t, e_t)
            o_acc2 = oap.tile([128, 128], BF16, tag="oacc")
            nc.gpsimd.tensor_add(o_acc2, o_acc, om)
            o_acc = o_acc2

        # ===== boundary: hT_enc = (o_sel/WS) * tanh(c_final) =====
        tce = work.tile([128, 2, BL], BF16, tag="tct")
        nc.scalar.activation(tce, c_prev, Tanh)
        hT_b = (hpool.tile([128, BL], BF16, tag="hT0", name="hTb0"),
                hpool.tile([128, BL], BF16, tag="hT1", name="hTb1"))
        for k in (0, 1):
            nc.vector.scalar_tensor_tensor(
                hT_b[k], o_acc[:, 64 * k:64 * k + 64], 1.0 / WS,
                tce[:, k, :], MUL, MUL)
        hT_prev = hT_b

        # ================= DECODER =================
        # y = out_W @ h + out_b: 2 matmuls per step accumulate into a
        # persistent PSUM bank (8 steps per group, k0 matmul opens each
        # column slice); one DVE tensor_scalar (+bias) + one DMA per group.
        psy_box = [None]
        pending_y = [None]
        last_dve = [None]

        def flush_y(after=None):
            """Deferred y matmuls: step j's out_W@h_j runs behind step j+1's
            gate matmuls in the PE FIFO so it never delays the next burst."""
            if pending_y[0] is None:
                return
            j, hT = pending_y[0]
            pending_y[0] = None
            g8 = j % 8
            ymms = [] if after is None else [after]
            if g8 == 0:
                psy_box[0] = tpp.tile([128, 512], f, tag="tp", name=f"psy{j}")
            psy = psy_box[0]
            for k in (0, 1):
                ymms.append(nc.tensor.matmul(
                    psy[:, 64 * g8:64 * g8 + 64], s_outwT[:, k, :],
                    hT[k], start=(k == 0 and g8 == 0), stop=(k == 1),
                    skip_group_check=True))
            chain(ymms)
            if g8 == 7 or j == t_dec - 1:
                cnt = g8 + 1
                y_sb = work.tile([D, 512], f, tag="ysb")
                ts = nc.vector.tensor_scalar_add(
                    y_sb[:, 0:64 * cnt], psy[0:D, 0:64 * cnt], s_outb)
                if last_dve[0] is not None:
                    add_dep_helper(ts.ins, last_dve[0].ins, sync=False,
                                   reason="defer y evac")
                nc.sync.dma_start(yt_d[:, j - g8 + 1:j + 2, :],
                                  y_sb[:, 0:64 * cnt])

        for j in range(t_dec):
            psA = gpool.tile([128, 256], f, tag="gA")
            psB = gbc.tile([128, 128], f, tag="gB")
            psC = gbc.tile([128, 128], f, tag="gC")
            if j == 0:
                last_mm = gate_mms(psA, psB, psC, s_whhdec,
                                   xlhs=s_wxdec, xrhs=s_x0p)
            else:
                last_mm = gate_mms(psA, psB, psC, s_wcomb, with_bias=True)
            flush_y(after=last_mm)
            _, last_mul = cell(psA, psB, psC)
            last_dve[0] = last_mul
            pending_y[0] = (j, hT_prev)
        flush_y()

    nc.compile()
    return nc


def _prep_host(inputs, t_enc=T, t_dec=TDEC):
    """Build per-core in_maps from full inputs (numpy)."""
    x = np.asarray(inputs["input_tensor"], np.float32)
    tgt = np.asarray(inputs["target_tensor"], np.float32)
    lens = np.asarray(inputs["lens"]).astype(np.int64)

    eWih = np.asarray(inputs["enc_Wih"], np.float32)
    eWhh = np.asarray(inputs["enc_Whh"], np.float32)
    eb = (np.asarray(inputs["enc_bih"], np.float32)
          + np.asarray(inputs["enc_bhh"], np.float32))
    dWih = np.asarray(inputs["dec_Wih"], np.float32)
    dWhh = np.asarray(inputs["dec_Whh"], np.float32)
    db = (np.asarray(inputs["dec_bih"], np.float32)
          + np.asarray(inputs["dec_bhh"], np.float32))
    oW = np.asarray(inputs["out_W"], np.float32)
    ob = np.asarray(inputs["out_b"], np.float32)

    wcomb_full = dWhh + dWih @ oW          # [G4, H]
    bcomb = db + dWih @ ob                 # [G4]

    def chunked_x(W, bias, freeze_big):
        # -> [66, 8, 128]: rows 0:64 x-weights^T (x WS), row 64 bias (x WS,
        # xp row 64 is 1/WS), row 65 freeze logits (xp row 65 is mbar)
        out = np.zeros((66, 8, 128), np.float32)
        for m, (r0, r1) in enumerate(CHUNK_ROWS):
            out[0:64, m, :] = W[r0:r1, :].T * WS
            out[64, m, :] = bias[r0:r1] * WS
            if freeze_big and m in (0, 1):
                out[65, m, :] = -BIG
            elif freeze_big and m in (2, 3):
                out[65, m, :] = BIG
        return out.astype(BF)

    def chunked_b(b):
        return np.stack([b[r0:r1] for (r0, r1) in CHUNK_ROWS])

    def chunked_h(W):
        # -> [128, 2, 8, 128], scaled by WS (h is carried at 1/WS)
        out = np.zeros((128, 2, 8, 128), np.float32)
        for m, (r0, r1) in enumerate(CHUNK_ROWS):
            for k in (0, 1):
                out[:, k, m, :] = W[r0:r1, 128 * k:128 * (k + 1)].T * WS
        return out.astype(BF)

    wxenc = chunked_x(eWih, eb, True)
    wxdec = chunked_x(dWih, db, False)
    whhenc = chunked_h(eWhh)
    whhdec = chunked_h(dWhh)
    wcomb = chunked_h(wcomb_full)
    # decoder steps >= 1 use the combined bias via rank-4 opener matmuls
    bc = chunked_b(bcomb)                  # [8, 128]
    biasA = bc[0:4].astype(BF)             # [4, 128]
    biasB = bc[6:8].astype(BF)             # [2, 128]
    biasC = bc[4:6].astype(BF)             # [2, 128]
    # blockones: rows 0-3 x cols 0-255 = 4x64 block-diag (bank A opener);
    # rows 4-5 x cols 256-383 = 2x64 block-diag (bank B/C opener)
    blockones = np.zeros((8, 512), np.float32)
    for m in range(4):
        blockones[m, 64 * m:64 * m + 64] = 1.0
    for m in range(2):
        blockones[4 + m, 256 + 64 * m:256 + 64 * m + 64] = 1.0
    blockones = blockones.astype(BF)
    # outwT padded to M=128 so FWL stays enabled; scaled by WS
    outwT = np.zeros((128, 2, 128), np.float32)
    outwT[:, :, 0:D] = (oW.T.reshape(2, 128, D).transpose(1, 0, 2)) * WS
    outwT = outwT.astype(BF)
    outb = ob[:, None].astype(np.float32).copy()

    tt = np.arange(t_enc)[None, :]
    in_maps = []
    for c in range(NCORES):
        b0 = c * BL
        xs = x[b0:b0 + BL, :t_enc, :]                # [BL,t,D]
        xp = np.empty((t_enc, 66, BL), np.float32)
        xp[:, 0:D, :] = xs.transpose(1, 2, 0) * (1.0 / WS)
        xp[:, D, :] = 1.0 / WS
        lc = lens[b0:b0 + BL]
        mbar = (tt >= lc[:, None]).astype(np.float32)   # [BL,t]
        xp[:, D + 1, :] = mbar.T
        efreeze = (tt == (lc[:, None] - 1)).astype(np.float32)  # [BL,t]
        # emask[t, p, 64k+b] = efreeze[b, t] for all p, k
        em = np.broadcast_to(efreeze.T[:, None, None, :],
                             (t_enc, 128, 2, BL)).reshape(t_enc, 128, 128)
        x0p = np.zeros((66, BL), np.float32)
        x0p[0:D, :] = tgt[b0:b0 + BL, 0, :].T * (1.0 / WS)
        x0p[D, :] = 1.0 / WS
        in_maps.append({
            "xp": np.ascontiguousarray(xp).astype(BF),
            "x0p": x0p.astype(BF),
            "wxenc": wxenc, "wxdec": wxdec,
            "whhenc": whhenc, "whhdec": whhdec, "wcomb": wcomb,
            "biasA": biasA, "biasB": biasB, "biasC": biasC,
            "blockones": blockones,
            "emask": np.ascontiguousarray(em).astype(BF),
            "outwT": outwT, "outb": outb,
        })
    return in_maps, lens


def kernel(**inputs) -> np.ndarray:
    global _PROGRAM, LAST_RESULTS
    if _PROGRAM is None:
        _PROGRAM = build_program()
    nc = _PROGRAM
    in_maps, lens = _prep_host(inputs)
    res = run_bass_kernel_spmd(nc, in_maps, core_ids=list(range(NCORES)))
    LAST_RESULTS = res
    out = np.zeros((B, T, D), np.float32)
    for c in range(NCORES):
        yt = res.results[c]["yt"]                      # [D, T, BL]
        out[c * BL:(c + 1) * BL] = yt.transpose(2, 1, 0)
    mask = (np.arange(T)[None, :] < lens[:, None])[:, :, None]
    out *= mask
    out[:, 0, :] = 0.0
    return out
